# revision 1
# baseline (speedup 1.0000x reference)
"""Causal self-attention on 8 Trainium2 NeuronCores.

Sharding (data + head parallel): core c handles batch b = c // 4 and the
4 heads [4g, 4g+4) where g = c % 4.  Each core projects q/k/v for its
heads (weights pre-sliced + pre-transposed on host), runs causal
attention, then the 4 cores of each batch AllGather the per-head
attention outputs (hd-major fp16) and each computes a disjoint
256-channel column slice of the output projection.

Pipeline notes:
- fp16 data path, fp32 PSUM accumulation.
- Attention runs in 512-column q-chunks; both heads of a pair share one
  score tile (bank-aligned halves) so each j-step needs a single exp.
- PSUM budget (8 banks): score tile 2 banks x 2 bufs, two 1-bank
  attention accumulators, one 2-bank projection accumulator.  The spare
  projection accumulator lets q/k projection for pair 1 and the output
  projection interleave with the ACT-bound attention loop, keeping the
  tensor engine dense (HAM stays un-throttled).
- AllGathers go per (pair, 1024 columns): 4 small collectives that
  overlap attention; gathered rows are prefetched to SBUF immediately.

Layouts per core:
  xT    (1024, 2048)  x[b].T                       (d on partitions)
  wqkT  (1024, 512)   [ (Wq[rows]/8).T | Wk[rows].T ]
  wvT   (1024, 256)   Wv[rows].T
  woT   (1024, 256)   Wo[rows].T with rows permuted to the AllGather
                      order: [pair p=0: rank r: heads 4r,4r+1] then
                      [pair p=1: rank r: heads 4r+2,4r+3]
  mask  (128, 128)    upper-triangular ones (k <= q)
  outT  (256, 2048)   out[b][:, cols].T
"""

import numpy as np

B, S, D, H = 2, 2048, 1024, 16
HD = D // H              # 64
NCORES = 8
GROUP = 4                # cores per batch
LHEADS = 4               # heads per core
LCH = LHEADS * HD        # 256 local channels
KT = D // 128            # 8 contraction tiles
ST = S // 128            # 16 sequence tiles
PAIRS = 2                # head pairs per core
CHUNK = 512              # q columns per attention pass
NCH = S // CHUNK         # 4
GATH = 1024              # columns per collective
NHB = S // GATH          # 2 gather blocks

_CACHE = {}


def _f16(a):
    return np.ascontiguousarray(a, dtype=np.float16)


def _build():
    import concourse.bacc as bacc
    import concourse.mybir as mybir
    import concourse.tile as tile

    f32 = mybir.dt.float32
    f16 = mybir.dt.float16
    Exp = mybir.ActivationFunctionType.Exp

    nc = bacc.Bacc(num_devices=NCORES)
    xT = nc.dram_tensor("xT", [D, S], f16, kind="ExternalInput")
    wqkT = nc.dram_tensor("wqkT", [D, 2 * LCH], f16, kind="ExternalInput")
    wvT = nc.dram_tensor("wvT", [D, LCH], f16, kind="ExternalInput")
    woT = nc.dram_tensor("woT", [D, LCH], f16, kind="ExternalInput")
    mask = nc.dram_tensor("mask", [128, 128], f16, kind="ExternalInput")
    outT = nc.dram_tensor("outT", [LCH, S], f32, kind="ExternalOutput")

    RG = [[0, 1, 2, 3], [4, 5, 6, 7]]

    with tile.TileContext(nc, num_cores=NCORES) as tc:
        with (
            tc.tile_pool(name="const", bufs=1) as const,
            tc.tile_pool(name="qkv", bufs=1) as qkv,
            tc.tile_pool(name="psum", bufs=1, space="PSUM") as psum,
            tc.tile_pool(name="dram", bufs=1, space="DRAM") as dram,
            tc.tile_pool(name="work", bufs=1) as work,
            tc.tile_pool(name="proj", bufs=1) as projp,
            tc.tile_pool(name="agp", bufs=1) as agp,
        ):
            mask_sb = const.tile([128, 128], f16)
            nc.sync.dma_start(mask_sb[:], mask[:])
            ones4 = const.tile([128, LHEADS], f32)
            nc.vector.memset(ones4[:], 1.0)

            cc_in = [[dram.tile([128, GATH], f16, name=f"ccin{p}{hb}")
                      for hb in range(NHB)] for p in range(PAIRS)]
            cc_out = [[dram.tile([GROUP * 128, GATH], f16, name=f"ccout{p}{hb}")
                       for hb in range(NHB)] for p in range(PAIRS)]

            qt = qkv.tile([128, PAIRS, S], f16)
            kt = qkv.tile([128, PAIRS, S], f16)
            v = qkv.tile([128, ST, LHEADS, 65], f16)

            # ---------------- input loads ----------------
            xt, wqk, wv = [], [], []
            for k in range(KT):
                tx = projp.tile([128, S], f16, name=f"xt{k}")
                nc.sync.dma_start(tx[:], xT[128 * k:128 * k + 128, :])
                xt.append(tx)
                tw = projp.tile([128, 2 * LCH], f16, name=f"wqk{k}")
                nc.sync.dma_start(tw[:], wqkT[128 * k:128 * k + 128, :])
                wqk.append(tw)
            for k in range(KT):
                tv = projp.tile([128, LCH], f16, name=f"wv{k}")
                nc.sync.dma_start(tv[:], wvT[128 * k:128 * k + 128, :])
                wv.append(tv)
            wo = projp.tile([128, KT, LCH], f16)
            nc.sync.dma_start(wo[:], woT[:].rearrange("(k p) n -> p k n", p=128))

            def qk_proj(m):
                # m: 0,1 = q pair 0/1; 2,3 = k pair 0/1
                dst = qt if m < 2 else kt
                for half in range(2):
                    pp = psum.tile([128, GATH], f32, tag="pp", name=f"qk{m}{half}")
                    for k in range(KT):
                        for c2 in range(2):
                            o = GATH * half + 512 * c2
                            nc.tensor.matmul(
                                pp[:, 512 * c2:512 * c2 + 512],
                                wqk[k][:, 128 * m:128 * m + 128],
                                xt[k][:, o:o + 512],
                                start=(k == 0), stop=(k == KT - 1))
                    nc.vector.tensor_copy(
                        dst[:, m % 2, GATH * half:GATH * half + GATH], pp[:])

            def v_proj(j):
                vps = psum.tile([128, LCH], f32, tag=("at" if j % 2 == 0 else "pp"),
                                name=f"v{j}")
                for k in range(KT):
                    nc.tensor.matmul(
                        vps[:], xt[k][:, 128 * j:128 * j + 128], wv[k][:],
                        start=(k == 0), stop=(k == KT - 1))
                nc.vector.tensor_copy(
                    v[:, j, :, 64:65], ones4[:].rearrange("p (h o) -> p h o", o=1))
                nc.vector.tensor_copy(
                    v[:, j, :, 0:64], vps[:].rearrange("p (h e) -> p h e", h=LHEADS))

            ag = {}
            att_last = [None]
            ccin_last = [None]

            def stage_chunk(p, c, attps):
                """Normalize chunk c's accumulators and ship to the collective
                buffer; AllGather once a 1024-col block is done."""
                hb, sub = c // 2, c % 2
                for h in range(2):
                    asb = work.tile([65, CHUNK], f32, tag=f"asb{h}", bufs=2,
                                    name=f"asb{p}{c}{h}")
                    nc.vector.tensor_copy(asb[:], attps[:, 512 * h:512 * h + CHUNK])
                    rc = work.tile([65, CHUNK], f32, tag="rc", bufs=2,
                                   name=f"rc{p}{c}{h}")
                    nc.vector.reciprocal(rc[64:65, :], asb[64:65, :])
                    rc0 = work.tile([1, CHUNK], f32, tag="rc0", bufs=2,
                                    name=f"rc0{p}{c}{h}")
                    nc.sync.dma_start(rc0[0:1, :], rc[64:65, :])
                    bc = work.tile([64, CHUNK], f32, tag="bc", bufs=2,
                                   name=f"bc{p}{c}{h}")
                    nc.gpsimd.partition_broadcast(bc[:], rc0[0:1, :])
                    ao = work.tile([64, CHUNK], f16, tag="ao", bufs=2,
                                   name=f"ao{p}{c}{h}")
                    nc.vector.tensor_mul(ao[:, :], asb[0:64, :], bc[:, :])
                    ccin_last[0] = nc.sync.dma_start(
                        cc_in[p][hb][64 * h:64 * h + 64,
                                     CHUNK * sub:CHUNK * sub + CHUNK], ao[:, :])
                if sub == 1:
                    nc.gpsimd.collective_compute(
                        "AllGather", mybir.AluOpType.bypass, replica_groups=RG,
                        ins=[cc_in[p][hb][:]], outs=[cc_out[p][hb][:]])

            def attn_chunk(p, c):
                q0 = CHUNK * c
                nj = 4 * c + 4
                attps = psum.tile([65, 2 * CHUNK], f32,
                                  tag=("at" if c % 2 == 0 else "pp"),
                                  name=f"att{p}{c}")
                for j in range(nj):
                    qs = max(q0, 128 * j)
                    n = q0 + CHUNK - qs
                    off = qs - q0
                    sc = psum.tile([128, 1024], f32, tag="sc", bufs=2,
                                   name=f"sc{p}{c}{j}")
                    for h in range(2):
                        pb = 64 * h
                        nc.tensor.matmul(
                            sc[:, 512 * h:512 * h + n],
                            kt[pb:pb + 64, p, 128 * j:128 * j + 128],
                            qt[pb:pb + 64, p, qs:qs + n],
                            start=True, stop=True)
                    ex = work.tile([128, 1024], f16, tag="ex", bufs=3,
                                   name=f"ex{p}{c}{j}")
                    nc.scalar.activation(
                        ex[:].rearrange("q (t x) -> q t x", t=2)[:, :, 0:n],
                        sc[:].rearrange("q (t x) -> q t x", t=2)[:, :, 0:n],
                        Exp)
                    if qs == 128 * j:  # diagonal tile: causal mask
                        for h in range(2):
                            nc.vector.tensor_mul(
                                ex[:, 512 * h:512 * h + 128],
                                ex[:, 512 * h:512 * h + 128], mask_sb[:])
                    for h in range(2):
                        att_last[0] = nc.tensor.matmul(
                            attps[:, 512 * h + off:512 * h + CHUNK],
                            v[:, j, 2 * p + h, :],
                            ex[:, 512 * h:512 * h + n],
                            start=(j == 0), stop=(j == nj - 1))
                stage_chunk(p, c, attps)

            def out_proj(hb):
                # The scheduler's cost model doesn't know collective latency;
                # pin the gather prefetch (and hence the matmuls) after the
                # last attention instruction so a long AllGather wait can't
                # stall the in-order engine streams mid-attention.
                for p in range(PAIRS):
                    for r in range(GROUP):
                        t = agp.tile([128, GATH], f16, name=f"ag{p}{hb}{r}")
                        dma = nc.sync.dma_start(
                            t[:], cc_out[p][hb][128 * r:128 * r + 128, :])
                        if ccin_last[0] is not None:
                            tile.add_dep_helper(
                                dma.ins, ccin_last[0].ins, sync=True,
                                reason="gather prefetch after all staging")
                        ag[(p, hb, r)] = t
                for ct in range(2):
                    pp = psum.tile([128, GATH], f32, tag="pp", name=f"op{hb}{ct}")
                    for k in range(KT):
                        for c2 in range(2):
                            nc.tensor.matmul(
                                pp[:, 512 * c2:512 * c2 + 512],
                                wo[:, k, 128 * ct:128 * ct + 128],
                                ag[(k // 4, hb, k % 4)][:, 512 * c2:512 * c2 + 512],
                                start=(k == 0), stop=(k == KT - 1))
                    ot = agp.tile([128, GATH], f32, tag=f"ot{ct}", bufs=2,
                                  name=f"ot{hb}{ct}")
                    nc.scalar.copy(ot[:], pp[:])
                    nc.sync.dma_start(
                        outT[128 * ct:128 * ct + 128,
                             GATH * hb:GATH * hb + GATH], ot[:])

            # ---------------- schedule ----------------
            qk_proj(0)            # pair-0 q
            qk_proj(2)            # pair-0 k
            for j in range(8):
                v_proj(j)
            attn_chunk(0, 0)
            attn_chunk(0, 1)
            for j in range(8, ST):
                v_proj(j)
            qk_proj(1)            # pair-1 q
            qk_proj(3)            # pair-1 k
            attn_chunk(0, 2)
            attn_chunk(0, 3)
            for c in range(NCH):
                attn_chunk(1, c)
            out_proj(0)
            out_proj(1)

    nc.compile()
    return nc


def _gather_perm():
    """d-channel permutation matching the AllGather layout."""
    perm = []
    for p in range(PAIRS):
        for r in range(GROUP):
            for h in range(2):
                head = 4 * r + 2 * p + h
                perm.extend(range(HD * head, HD * head + HD))
    return np.array(perm)


def _shard_inputs(x, Wq, Wk, Wv, Wo):
    x = np.asarray(x, dtype=np.float32)
    Wq = np.asarray(Wq, dtype=np.float32)
    Wk = np.asarray(Wk, dtype=np.float32)
    Wv = np.asarray(Wv, dtype=np.float32)
    Wo = np.asarray(Wo, dtype=np.float32)
    mask = np.triu(np.ones((128, 128), dtype=np.float16))
    perm = _gather_perm()
    in_maps = []
    for c in range(NCORES):
        b, g = c // GROUP, c % GROUP
        rows = slice(LCH * g, LCH * g + LCH)
        in_maps.append({
            "xT": _f16(x[b].T),
            "wqkT": _f16(np.concatenate([Wq[rows] / 8.0, Wk[rows]], axis=0).T),
            "wvT": _f16(Wv[rows].T),
            "woT": _f16(Wo[rows].T[perm, :]),
            "mask": mask,
        })
    return in_maps


def kernel(x, Wq, Wk, Wv, Wo):
    from concourse.bass_utils import run_bass_kernel_spmd

    if "nc" not in _CACHE:
        _CACHE["nc"] = _build()
    nc = _CACHE["nc"]
    in_maps = _shard_inputs(x, Wq, Wk, Wv, Wo)
    res = run_bass_kernel_spmd(nc, in_maps, core_ids=list(range(NCORES)))
    _CACHE["last_results"] = res
    out = np.empty((B, S, D), dtype=np.float32)
    for c in range(NCORES):
        b, g = c // GROUP, c % GROUP
        out[b][:, LCH * g:LCH * g + LCH] = res.results[c]["outT"].T
    return out



# revision 9
# speedup vs baseline: 1.2413x; 1.2413x over previous
"""Causal self-attention on 8 Trainium2 NeuronCores.

Sharding: core c handles batch b = c // 4 and the 4 heads
[4g, 4g+4) with g = c % 4 (weights pre-sliced + pre-transposed on
host).  Each core projects q/k/v for its heads, runs causal attention
in 512-column q-chunks (pair-interleaved: c0p0, c0p1, c1p0, ...),
then the attention outputs are resharded with one 8-rank AllToAll per
head pair: shard j = this core's pair channels for tokens
[256j, 256j+256) of its batch, so core j ends up with ALL 1024
attention channels for a 256-token slice of BOTH batches and computes
the full output projection for that slice locally (no AllGather, 4x
less wire traffic, and the first A2A overlaps the last chunk).

Pipeline notes:
- fp16 data path, fp32 PSUM accumulation.
- x / Wq / Wk are loaded per (k-tile, token-block) so the first
  projection matmul can start ~3us in; attention chunk 0 only needs
  token block 0.
- Softmax denominators: ones-row in the attended matmul (row 64 of
  attps); 1/den via reciprocal_approx_fast straight off the PSUM row
  (the iterative-divide DVE reciprocal is 8 cyc/elem on one lane =
  4us/chunk; approx_fast is a normal pipelined op, ~51 ULP which is
  far below the fp16 data path noise).
- PSUM budget (8 banks): score tile [128,1024] x 2 bufs (4), one
  shared attps [65,1024] (2), two 1-bank projection accumulators that
  qk/v/out projections cycle through.
- Out projection k-groups are split per pair so the pair-0 half can
  accumulate while the pair-1 AllToAll is still in flight.

Per-core layouts:
  xT    (1024, 2048)  x[b].T                  (d on partitions)
  wqkT  (1024, 512)   [ (Wq[rows]/8).T | Wk[rows].T ]
  wvT   (1024, 256)   Wv[rows].T
  woT   (1024, 1024)  Wo.T with input-channel rows permuted to the
                      A2A order: [pair p: rank r: heads 4r+2p+{0,1}]
  mask  (128, 128)    upper-triangular ones (k <= q)
  outT  (1024, 512)   out channels x [b0 toks 256c..  | b1 toks ...]
"""

import numpy as np

B, S, D, H = 2, 2048, 1024, 16
HD = D // H              # 64
NCORES = 8
GROUP = 4                # cores per batch
LHEADS = 4               # heads per core
LCH = LHEADS * HD        # 256 local channels
KT = D // 128            # 8 contraction tiles
ST = S // 128            # 16 sequence tiles
PAIRS = 2                # head pairs per core
CHUNK = 512              # q columns per attention pass
NCH = S // CHUNK         # 4
TOK = S // NCORES        # 256 output tokens per core (per batch)

_CACHE = {}
DEBUG = False            # adds cc_in/cc_out dumps as extra outputs


def _f16(a):
    return np.ascontiguousarray(a, dtype=np.float16)


def _build():
    import concourse.bacc as bacc
    import concourse.mybir as mybir
    import concourse.tile as tile

    f32 = mybir.dt.float32
    f16 = mybir.dt.float16
    Exp = mybir.ActivationFunctionType.Exp

    nc = bacc.Bacc(num_devices=NCORES)
    xT = nc.dram_tensor("xT", [D, S], f16, kind="ExternalInput")
    wqkT = nc.dram_tensor("wqkT", [D, 2 * LCH], f16, kind="ExternalInput")
    wvT = nc.dram_tensor("wvT", [D, LCH], f16, kind="ExternalInput")
    woT = nc.dram_tensor("woT", [D, D], f16, kind="ExternalInput")
    mask = nc.dram_tensor("mask", [128, 128], f16, kind="ExternalInput")
    outT = nc.dram_tensor("outT", [D, 2 * TOK], f32, kind="ExternalOutput")

    RG = [list(range(NCORES))]

    with tile.TileContext(nc, num_cores=NCORES) as tc:
        with (
            tc.tile_pool(name="const", bufs=1) as const,
            tc.tile_pool(name="qkv", bufs=1) as qkv,
            tc.tile_pool(name="psum", bufs=1, space="PSUM") as psum,
            tc.tile_pool(name="dram", bufs=1, space="DRAM") as dram,
            tc.tile_pool(name="work", bufs=1) as work,
            tc.tile_pool(name="proj", bufs=1) as projp,
            tc.tile_pool(name="agp", bufs=1) as agp,
        ):
            mask_sb = const.tile([128, 128], f16)
            nc.sync.dma_start(mask_sb[:], mask[:])

            cc_in = [dram.tile([NCORES * 128, TOK], f16, name=f"ccin{p}")
                     for p in range(PAIRS)]
            cc_out = [dram.tile([NCORES * 128, TOK], f16, name=f"ccout{p}")
                      for p in range(PAIRS)]

            qt = qkv.tile([128, PAIRS, S], f16)
            kt = qkv.tile([128, PAIRS, S], f16)
            v = qkv.tile([128, ST, LHEADS, 65], f16)
            nc.vector.memset(v[:, :, :, 64:65], 1.0)

            # ---------------- input loads ----------------
            # x per (k-tile, token-block) so early matmuls start early.
            wqk, wv = [], []
            for k in range(KT):
                tw = projp.tile([128, 2 * LCH], f16, name=f"wqk{k}")
                nc.sync.dma_start(tw[:], wqkT[128 * k:128 * k + 128, :])
                wqk.append(tw)
            xt = [[None] * NCH for _ in range(KT)]
            for t in range(NCH):
                for k in range(KT):
                    tx = projp.tile([128, CHUNK], f16, name=f"xt{k}_{t}")
                    nc.sync.dma_start(
                        tx[:], xT[128 * k:128 * k + 128,
                                  CHUNK * t:CHUNK * t + CHUNK])
                    xt[k][t] = tx
                if t == 0:
                    for k in range(KT):
                        tv = projp.tile([128, LCH], f16, name=f"wv{k}")
                        nc.sync.dma_start(tv[:], wvT[128 * k:128 * k + 128, :])
                        wv.append(tv)
            wo = projp.tile([128, KT, D], f16)
            nc.sync.dma_start(wo[:], woT[:].rearrange("(k p) n -> p k n", p=128))

            # Preload the exp table set off the critical path.
            warm = const.tile([1, 16], f16)
            nc.scalar.activation(warm[:], mask_sb[0:1, 0:16], Exp)

            def qk_proj(m, t):
                # m: 0,1 = q pair 0/1; 2,3 = k pair 0/1; t: token block
                dst = qt if m < 2 else kt
                pp = psum.tile([128, CHUNK], f32, tag="pp0", name=f"qk{m}{t}")
                for k in range(KT):
                    nc.tensor.matmul(
                        pp[:], wqk[k][:, 128 * m:128 * m + 128], xt[k][t][:],
                        start=(k == 0), stop=(k == KT - 1))
                nc.vector.tensor_copy(
                    dst[:, m % 2, CHUNK * t:CHUNK * t + CHUNK], pp[:])

            def v_proj(j):
                t = j // 4
                o = 128 * (j % 4)
                vps = psum.tile([128, LCH], f32, tag="pp1", name=f"v{j}")
                for k in range(KT):
                    nc.tensor.matmul(
                        vps[:], xt[k][t][:, o:o + 128], wv[k][:],
                        start=(k == 0), stop=(k == KT - 1))
                nc.vector.tensor_copy(
                    v[:, j, :, 0:64], vps[:].rearrange("p (h e) -> p h e", h=LHEADS))

            stage_last = [None]
            dbg_tiles = [None, None, None]

            def attn_chunk(p, c):
                q0 = CHUNK * c
                nj = 4 * c + 4
                attps = psum.tile([65, 2 * CHUNK], f32, tag="at",
                                  name=f"att{p}{c}")
                for j in range(nj):
                    qs = max(q0, 128 * j)
                    n = q0 + CHUNK - qs
                    off = qs - q0
                    sc = psum.tile([128, 1024], f32, tag="sc", bufs=2,
                                   name=f"sc{p}{c}{j}")
                    for h in range(2):
                        pb = 64 * h
                        nc.tensor.matmul(
                            sc[:, 512 * h:512 * h + n],
                            kt[pb:pb + 64, p, 128 * j:128 * j + 128],
                            qt[pb:pb + 64, p, qs:qs + n],
                            start=True, stop=True)
                    ex = work.tile([128, 1024], f16, tag="ex", bufs=3,
                                   name=f"ex{p}{c}{j}")
                    nc.scalar.activation(
                        ex[:].rearrange("q (t x) -> q t x", t=2)[:, :, 0:n],
                        sc[:].rearrange("q (t x) -> q t x", t=2)[:, :, 0:n],
                        Exp)
                    if qs == 128 * j:  # diagonal tile: causal mask
                        for h in range(2):
                            nc.vector.tensor_mul(
                                ex[:, 512 * h:512 * h + 128],
                                ex[:, 512 * h:512 * h + 128], mask_sb[:])
                    for h in range(2):
                        nc.tensor.matmul(
                            attps[:, 512 * h + off:512 * h + CHUNK],
                            v[:, j, 2 * p + h, :],
                            ex[:, 512 * h:512 * h + n],
                            start=(j == 0), stop=(j == nj - 1))
                # normalize + stage into the AllToAll buffer
                den = work.tile([1, 2 * CHUNK], f32, tag="den", bufs=2,
                                name=f"den{p}{c}")
                nc.vector.tensor_copy(den[:], attps[64:65, :])
                rc0 = work.tile([1, 2 * CHUNK], f32, tag="rc0", bufs=2,
                                name=f"rc0{p}{c}")
                nc.vector.reciprocal_approx_fast(rc0[:], den[:])
                bc = work.tile([64, 2 * CHUNK], f32, tag="bc", bufs=2,
                               name=f"bc{p}{c}")
                nc.gpsimd.partition_broadcast(bc[:], rc0[0:1, :])
                dbg_tiles[0], dbg_tiles[1], dbg_tiles[2] = den, rc0, bc
                for h in range(2):
                    ao = work.tile([64, CHUNK], f16, tag=f"ao{h}", bufs=2,
                                   name=f"ao{p}{c}{h}")
                    nc.vector.tensor_mul(
                        ao[:, :], attps[0:64, 512 * h:512 * h + CHUNK],
                        bc[:, 512 * h:512 * h + CHUNK])
                    # shard s = 2c + sub holds tokens [256s, 256s+256)
                    for sub in range(2):
                        r0 = 128 * (2 * c + sub) + 64 * h
                        stage_last[0] = nc.sync.dma_start(
                            cc_in[p][r0:r0 + 64, :],
                            ao[:, TOK * sub:TOK * sub + TOK])

            def a2a(p):
                nc.gpsimd.collective_compute(
                    "AllToAll", mybir.AluOpType.bypass, replica_groups=RG,
                    ins=[cc_in[p][:]], outs=[cc_out[p][:]])

            ag = {}

            def ag_prefetch(p):
                # ag[(p, r)] = [128 chans, b0 toks | b1 toks]
                for r in range(GROUP):
                    tile_ = agp.tile([128, 2 * TOK], f16, name=f"ag{p}{r}")
                    for b in range(2):
                        s = 4 * b + r
                        dma = nc.sync.dma_start(
                            tile_[:, TOK * b:TOK * b + TOK],
                            cc_out[p][128 * s:128 * s + 128, :])
                        if stage_last[0] is not None:
                            tile.add_dep_helper(
                                dma.ins, stage_last[0].ins, sync=True,
                                reason="gather prefetch after all staging")
                    ag[(p, r)] = tile_

            def out_proj():
                # 8 output-channel row tiles; contraction k-groups are
                # (pair, rank) so the pair-0 half can run before the
                # pair-1 AllToAll lands.
                for o in range(KT):
                    pp = psum.tile([128, 2 * TOK], f32,
                                   tag=("pp0" if o % 2 == 0 else "pp1"),
                                   name=f"op{o}")
                    for p in range(PAIRS):
                        for r in range(GROUP):
                            k = 4 * p + r
                            nc.tensor.matmul(
                                pp[:], wo[:, k, 128 * o:128 * o + 128],
                                ag[(p, r)][:],
                                start=(k == 0), stop=(k == 2 * GROUP - 1))
                    ot = agp.tile([128, 2 * TOK], f32, tag=f"ot{o % 2}",
                                  bufs=2, name=f"ot{o}")
                    if o % 2 == 0:
                        nc.scalar.copy(ot[:], pp[:])
                    else:
                        nc.vector.tensor_copy(ot[:], pp[:])
                    nc.sync.dma_start(
                        outT[128 * o:128 * o + 128, :], ot[:])

            # ---------------- schedule ----------------
            qk_proj(0, 0)
            qk_proj(2, 0)
            for j in range(4):
                v_proj(j)
            attn_chunk(0, 0)
            qk_proj(1, 0)
            qk_proj(3, 0)
            attn_chunk(1, 0)
            for c in range(1, NCH):
                for m in range(4):
                    qk_proj(m, c)
                for j in range(4 * c, 4 * c + 4):
                    v_proj(j)
                attn_chunk(0, c)
                if c == NCH - 1:
                    a2a(0)
                attn_chunk(1, c)
                if c == NCH - 1:
                    a2a(1)
            ag_prefetch(0)
            ag_prefetch(1)
            out_proj()
            if DEBUG:
                dr = nc.dram_tensor("dbg_rc", [3, 2 * CHUNK], f32,
                                    kind="ExternalOutput")
                nc.sync.dma_start(dr[0:1, :], dbg_tiles[0][:])
                nc.sync.dma_start(dr[1:2, :], dbg_tiles[1][:])
                nc.sync.dma_start(dr[2:3, :], dbg_tiles[2][0:1, :])
                for p in range(PAIRS):
                    di = nc.dram_tensor(f"dbg_in{p}", [NCORES * 128, TOK],
                                        f16, kind="ExternalOutput")
                    do = nc.dram_tensor(f"dbg_out{p}", [NCORES * 128, TOK],
                                        f16, kind="ExternalOutput")
                    nc.sync.dma_start(di[:], cc_in[p][:])
                    nc.sync.dma_start(do[:], cc_out[p][:])

    nc.compile()
    return nc


def _perm():
    """d-channel permutation matching the AllToAll layout."""
    perm = []
    for p in range(PAIRS):
        for r in range(GROUP):
            for h in range(2):
                head = 4 * r + 2 * p + h
                perm.extend(range(HD * head, HD * head + HD))
    return np.array(perm)


def _shard_inputs(x, Wq, Wk, Wv, Wo):
    x = np.asarray(x, dtype=np.float32)
    Wq = np.asarray(Wq, dtype=np.float32)
    Wk = np.asarray(Wk, dtype=np.float32)
    Wv = np.asarray(Wv, dtype=np.float32)
    Wo = np.asarray(Wo, dtype=np.float32)
    mask = np.triu(np.ones((128, 128), dtype=np.float16))
    woT = _f16(Wo.T[_perm(), :])
    in_maps = []
    for c in range(NCORES):
        b, g = c // GROUP, c % GROUP
        rows = slice(LCH * g, LCH * g + LCH)
        in_maps.append({
            "xT": _f16(x[b].T),
            "wqkT": _f16(np.concatenate([Wq[rows] / 8.0, Wk[rows]], axis=0).T),
            "wvT": _f16(Wv[rows].T),
            "woT": woT,
            "mask": mask,
        })
    return in_maps


def kernel(x, Wq, Wk, Wv, Wo):
    from concourse.bass_utils import run_bass_kernel_spmd

    if "nc" not in _CACHE:
        _CACHE["nc"] = _build()
    nc = _CACHE["nc"]
    in_maps = _shard_inputs(x, Wq, Wk, Wv, Wo)
    res = run_bass_kernel_spmd(nc, in_maps, core_ids=list(range(NCORES)))
    _CACHE["last_results"] = res
    out = np.empty((B, S, D), dtype=np.float32)
    for c in range(NCORES):
        o = res.results[c]["outT"]
        out[0][TOK * c:TOK * c + TOK, :] = o[:, 0:TOK].T
        out[1][TOK * c:TOK * c + TOK, :] = o[:, TOK:2 * TOK].T
    return out
